# revision 20
# baseline (speedup 1.0000x reference)
"""KAN expert kernel for Trainium2 (8 NeuronCores, data-parallel over batch).

Math: out[b,j] = sum_{i,g} basis_g(x[b,i]) * coeff[i,j,g] * scaling[i,j]
with cubic B-spline basis on the uniform extended grid g_m = -1 + 0.4*m.

Truncated-power identity: basis_g(x) = (1/(6h^3)) sum_r w_r relu(x-g_{g+r})^3
with w = [1,-4,6,-4,1]; for x in [-1,1) only features m=0..4 are nonzero and
the binomial combine folds into the weights on the host:
    C'[m,i,j] = (1/(6h^3)) * sum_g w[m-g] * coeff[i,j,g] * scaling[i,j]
so each core computes Q_m = relu(x - g_m)^3 (m=0..4) and one
[512b x 2560k] @ [2560k x 512j] matmul accumulated in PSUM.

Perf/precision design (v3):
- The contraction runs in fp16 on the PE (1 cycle/row vs 4 for fp32).
- The truncated-power basis cancels internally by ~100x, so fp16 noise on
  Q is amplified: Q must carry a SINGLE fp16 rounding. r = max(x-g,0) and
  s = r^2 are computed in fp32 (r on Pool/DVE, s on ACT), then
  q = (x-g)*s is one scalar_tensor_tensor on DVE with fp16 output — the
  relu clamp rides on s (s=0 where r=0). Measured config error ~7e-3 rel.
- DMA: each queue tops out near ~110 B/ns, so traffic is spread over the
  three DGE queues (Pool SWDGE, SP HWDGE, ACT HWDGE), with host-side
  swizzles making every transfer one contiguous run per partition.
  Weights stream in m-groups ordered to meet the PE's consumption
  deadlines; x lands first in two halves.
- PE HAM warmup: ~9 junk matmuls right after boot hold the PE busy so the
  clock gate reaches 2.4 GHz before the real stream starts.
- Each PSUM bank is evicted (DVE copy, fp16) and shipped as soon as its
  accumulation closes; out-DMAs bump a shared semaphore that replaces the
  end-of-block drains' multi-queue waits (walrus allows 1 wait there).
"""

import numpy as np

BATCH = 4096
IN_DIM = 512
OUT_DIM = 512
GRID_SIZE = 5
K = 3
N_CORES = 8
P = 128
NM = 5                      # relu^3 feature channels
BC = BATCH // N_CORES       # 512 batch rows per core
NIC = IN_DIM // P           # 4 input-dim chunks
NBC = BC // P               # 4 psum banks (batch chunks)
FW = NIC * BC               # 2048 free columns of X/Q tiles
HW = FW // 2                # x DMA halves
QW = FW // 4                # m=0 feature quarters (= one ic chunk)
GW = NIC * OUT_DIM          # 2048: free width of one weight m-group
N_WARM = 9                  # PE warmup matmuls

_W_BINOM = np.array([1.0, -4.0, 6.0, -4.0, 1.0])

_cached = {}


def _grid_f32():
    h = 2.0 / GRID_SIZE
    return np.float32(-1.0 + h * np.arange(GRID_SIZE + 2 * K + 1))


def _build_nc(debug_waits=False):
    import bass_rust as _br
    import concourse.bass as bass
    import concourse.mybir as mybir
    from concourse.tile import TileContext
    from concourse.bass import _add_dep_helper

    dt = mybir.dt
    alu = mybir.AluOpType
    act_fn = mybir.ActivationFunctionType
    grid = _grid_f32()

    nc = bass.Bass()
    xs = nc.dram_tensor("xs", [P, FW], dt.float16, kind="ExternalInput")
    ws = nc.dram_tensor("ws", [P, NM * GW], dt.float16, kind="ExternalInput")
    outd = nc.dram_tensor("outd", [P, NBC * OUT_DIM], dt.float16,
                          kind="ExternalOutput")
    # Shared completion semaphore for the out-DMAs: end-of-block drains may
    # carry at most one sync wait through walrus; osem >= 64 transitively
    # implies everything upstream (evicts <- matmuls <- features <- DMAs).
    osem = nc.alloc_semaphore("outdone")

    with TileContext(nc) as tc:
        with tc.tile_pool(name="main", bufs=1) as pool, \
             tc.tile_pool(name="psum", bufs=1, space="PSUM") as psum_pool:
            X = pool.tile([P, FW], dt.float16, tag="X")
            Wt = pool.tile([P, NM * GW], dt.float16, tag="W")
            warm = pool.tile([P, OUT_DIM], dt.float16, tag="warm")
            O = pool.tile([P, NBC * OUT_DIM], dt.float16, tag="O")
            scratch = pool.tile([1, 1], dt.float32, tag="scratch")

            nc.gpsimd.memset(warm[:], 0.0)

            # --- DMA issues, ordered per queue to meet PE deadlines ---
            # ACT queue: W m0(ic0) first (first matmul), then m0 rest, m2.
            nc.scalar.dma_start(out=Wt[:, 0:OUT_DIM], in_=ws[:, 0:OUT_DIM])
            nc.scalar.dma_start(out=Wt[:, OUT_DIM:GW], in_=ws[:, OUT_DIM:GW])
            # Pool queue: x half 0, then W m4.
            nc.gpsimd.dma_start(out=X[:, 0:HW], in_=xs[:, 0:HW])
            # SP queue: x half 1, then W m1, W m3.
            nc.sync.dma_start(out=X[:, HW:FW], in_=xs[:, HW:FW])
            nc.sync.dma_start(out=Wt[:, 1 * GW:2 * GW], in_=ws[:, 1 * GW:2 * GW])
            nc.scalar.dma_start(out=Wt[:, 2 * GW:3 * GW], in_=ws[:, 2 * GW:3 * GW])
            nc.sync.dma_start(out=Wt[:, 3 * GW:4 * GW], in_=ws[:, 3 * GW:4 * GW])
            nc.gpsimd.dma_start(out=Wt[:, 4 * GW:5 * GW], in_=ws[:, 4 * GW:5 * GW])

            # --- PE warmup: junk matmuls to lift the HAM clock gate ---
            ps_warm = psum_pool.tile([P, OUT_DIM], dt.float32, tag="psw",
                                     name="psw")
            for _ in range(N_WARM):
                nc.tensor.matmul(ps_warm[:], warm[:, 0:P], warm[:],
                                 start=True, stop=True)

            # --- features: r = max(x-g,0) fp32; s = r^2 fp32 (ACT);
            #     q = (x-g)*s -> single-rounded fp16 (DVE STT).
            # ACT "probes" of each W chunk precede the group's squares in
            # ACT program order, so a matmul's single wait on its q's DVE
            # tick transitively guarantees its weights landed.
            def probe(off):
                return nc.scalar.activation(scratch[0:1, 0:1],
                                            Wt[0:1, off:off + 1], act_fn.Copy)

            R = [pool.tile([P, FW], dt.float32, tag=f"r{m}", name=f"r{m}")
                 for m in range(NM)]
            S = [pool.tile([P, FW], dt.float32, tag=f"s{m}", name=f"s{m}")
                 for m in range(NM)]
            Q = [pool.tile([P, FW], dt.float16, tag=f"q{m}", name=f"q{m}")
                 for m in range(NM)]

            # m = 0: quarter (per-ic) granularity for the fastest first chain
            g0 = float(grid[0])
            for icq in range(4):
                sl = slice(icq * QW, (icq + 1) * QW)
                pr = probe(0 if icq == 0 else OUT_DIM) if icq <= 1 else None
                nc.vector.tensor_scalar(R[0][:, sl], X[:, sl], g0, 0.0,
                                        alu.subtract, alu.max)
                sq = nc.scalar.activation(S[0][:, sl], R[0][:, sl],
                                          act_fn.Square)
                if pr is not None:
                    _add_dep_helper(sq.ins, pr.ins, sync=False,
                                    reason="W probe before square")
                nc.vector.scalar_tensor_tensor(Q[0][:, sl], X[:, sl], g0,
                                               S[0][:, sl],
                                               alu.subtract, alu.mult)
            # m = 1..4: halves; r on Pool to keep DVE under the PE rate
            for m in range(1, NM):
                gm = float(grid[m])
                pr = probe(m * GW)
                for h in range(2):
                    sl = slice(h * HW, (h + 1) * HW)
                    nc.gpsimd.tensor_scalar(R[m][:, sl], X[:, sl], gm, 0.0,
                                            alu.subtract, alu.max)
                    sq = nc.scalar.activation(S[m][:, sl], R[m][:, sl],
                                              act_fn.Square)
                    if h == 0:
                        _add_dep_helper(sq.ins, pr.ins, sync=False,
                                        reason="W probe before square")
                    nc.vector.scalar_tensor_tensor(Q[m][:, sl], X[:, sl], gm,
                                                   S[m][:, sl],
                                                   alu.subtract, alu.mult)

            # --- matmuls: psum[bc] += Q_m[:, ic-slice].T @ W[m, ic] ---
            psums = [psum_pool.tile([P, OUT_DIM], dt.float32, tag=f"ps{b}",
                                    name=f"ps{b}")
                     for b in range(NBC)]
            out_dmas = []
            # then_inc is HWDGE-only, so out-DMAs ride the SP/ACT queues
            out_eng = [nc.sync, nc.scalar, nc.sync, nc.scalar]
            n_k = NM * NIC
            for m in range(NM):
                for ic in range(NIC):
                    kc = m * NIC + ic
                    w_sl = Wt[:, kc * OUT_DIM:(kc + 1) * OUT_DIM]
                    for bc in range(NBC):
                        lhsT = Q[m][:, ic * BC + bc * P: ic * BC + (bc + 1) * P]
                        nc.tensor.matmul(psums[bc][:], lhsT, w_sl,
                                         start=(kc == 0), stop=(kc == n_k - 1))
                        if kc == n_k - 1:
                            o_sl = O[:, bc * OUT_DIM:(bc + 1) * OUT_DIM]
                            nc.vector.tensor_copy(out=o_sl, in_=psums[bc][:])
                            od = out_eng[bc].dma_start(
                                out=outd[:, bc * OUT_DIM:(bc + 1) * OUT_DIM],
                                in_=o_sl).then_inc(osem, 16)
                            out_dmas.append(od)

    # Walrus rejects >1 sync wait per compute instruction. Prune
    # provably-redundant waits:
    #  - same-engine waits (every engine executes its stream in order),
    #  - matmul: keep only the DVE (q) wait — the probe->square->q chain
    #    transitively guarantees the weights landed,
    #  - DVE ops with an ACT wait: drop DMA waits (X landed upstream of s),
    #  - DMA copies: queue-order WAR waits are guaranteed by the ring,
    #  - end-of-block drains: replace DMA-queue waits with one wait on
    #    osem >= 64, which transitively implies the rest.
    eng2sem = {"EngineType.DVE": "DVE_",
               "EngineType.Activation": "Activation_",
               "EngineType.PE": "PE_",
               "EngineType.SP": "SP_",
               "EngineType.Pool": "Pool_"}
    dma_pref = ("DMASW", "DMAHW")
    multi = []
    for blk in nc.m.functions[0].blocks:
        for inst in blk.instructions:
            si = inst.sync_info
            if si is None or not si.on_wait:
                continue
            tn = type(inst).__name__
            eng = str(inst.engine)
            pref = eng2sem.get(eng)
            keep = [w for w in si.on_wait
                    if pref is None
                    or not (w.ant_name or "").startswith(pref)]
            if tn == "InstMatmult" or tn == "InstLdweights":
                dve = [w for w in keep
                       if (w.ant_name or "").startswith("DVE_")]
                pool_w = [w for w in keep
                          if (w.ant_name or "").startswith("Pool_")]
                if dve:
                    keep = dve
                elif pool_w:
                    keep = pool_w
            elif eng == "EngineType.DVE" and any(
                    (w.ant_name or "").startswith("Activation_")
                    for w in keep):
                keep = [w for w in keep
                        if not (w.ant_name or "").startswith(dma_pref)]
            if tn == "InstDMACopy":
                nq = [w for w in keep
                      if not (w.ant_name or "").startswith(dma_pref)]
                if nq:
                    keep = nq
            if tn == "InstDrain":
                if any((w.ant_name or "").startswith(dma_pref)
                       for w in keep):
                    keep = [_br.SyncWait(
                        sync_type="semaphore", id=osem.num,
                        ant_name=osem.name, wait_mode="sem-ge-imm",
                        wait_value=16 * NBC, wait_reg=None)]
            if len(keep) != len(si.on_wait):
                si.on_wait = keep
            if len(keep) > 1 and tn != "InstDrain":
                multi.append((inst.name, tn, eng,
                              [w.ant_name for w in keep]))
    if debug_waits and multi:
        for mm in multi:
            print("MULTIWAIT:", mm)
    assert debug_waits or not multi, \
        f"multi-wait compute instructions remain: {multi}"
    return nc


def _prep_weights(spline_coeff, spline_scaling, dtype):
    # C'[m,i,j] = (1/(6h^3)) * sum_g w[m-g] * coeff[i,j,g] * scaling[i,j]
    h = 2.0 / GRID_SIZE
    c = (spline_coeff.astype(np.float64)
         * spline_scaling.astype(np.float64)[:, :, None])  # [i, j, g]
    cp = np.zeros((NM, IN_DIM, OUT_DIM), np.float64)
    for m in range(NM):
        for g in range(max(0, m - 4), m + 1):
            cp[m] += _W_BINOM[m - g] * c[:, :, g]
    cp *= 1.0 / (6.0 * h ** 3)
    # swizzle [m, i=ic*P+p, j] -> [p, (m, ic, j)]
    wsw = cp.reshape(NM, NIC, P, OUT_DIM).transpose(2, 0, 1, 3)
    return np.ascontiguousarray(wsw.reshape(P, NM * GW).astype(dtype))


def _run(inputs, trace=False, mm_dtype_name="float16"):
    from concourse.bass_utils import run_bass_kernel_spmd

    if "nc" not in _cached:
        _cached["nc"] = _build_nc()
    nc = _cached["nc"]

    x = np.asarray(inputs["x"], np.float32)
    wsw = _prep_weights(np.asarray(inputs["spline_coeff"]),
                        np.asarray(inputs["spline_scaling"]), np.float16)
    in_maps = []
    for c in range(N_CORES):
        xc = x[c * BC:(c + 1) * BC, :]  # [b, i]
        xsw = np.ascontiguousarray(
            xc.T.reshape(NIC, P, BC).transpose(1, 0, 2)
            .reshape(P, FW).astype(np.float16))
        in_maps.append({"xs": xsw, "ws": wsw})
    res = run_bass_kernel_spmd(nc, in_maps, list(range(N_CORES)),
                               trace=trace)
    parts = []
    for c in range(N_CORES):
        od = res.results[c]["outd"]  # [P, NBC*OUT_DIM] fp16
        parts.append(od.reshape(P, NBC, OUT_DIM).transpose(1, 0, 2)
                     .reshape(BC, OUT_DIM))
    return np.concatenate(parts, axis=0).astype(np.float32), res


def kernel(**inputs):
    outp, _ = _run(inputs, trace=False)
    return outp
